# revision 7
# baseline (speedup 1.0000x reference)
"""DecoderRNNWithAttention single-step kernel for 8 Trainium2 NeuronCores.

Sharding: data-parallel over batch (2048 -> 8 x 256 rows, params replicated).
Each core: embedding (host-gathered) -> Bahdanau attention over 64 slots ->
GRU cell (H=512) -> vocab projection (V=32000) + log_softmax.

log_softmax trick: store p = exp(logit - C) as fp16 in SBUF while the
projection streams (ACT Exp with accum_out gives per-tile row-sums for free),
then y_cap = Ln(p * (1/S)) with a per-partition scale in one ACT op per chunk.
"""

import math

import numpy as np
import ml_dtypes

import concourse.bass as bass
import concourse.tile as tile
import concourse.mybir as mybir
from concourse.bass_utils import run_bass_kernel_spmd
from concourse.masks import make_identity

# Problem shape (hardcoded per contract)
B, L, H, E, V = 2048, 64, 512, 256, 32000
EH = E + H            # 768
G3 = 3 * H            # 1536
NCORES = 8
BC = B // NCORES      # 256 rows per core
P = 128
NB = BC // P          # 2 row blocks per core
VP = 32768            # vocab padded to 64 x 512
VT = 512              # vocab tile (one PSUM bank of f32)
NCH = VP // (2 * VT)  # 32 chunks of 2 vocab tiles
VPAD = VP - V         # 768 padded columns
EXP_C = 5.0           # fixed shift for exp()/fp16 range placement

F32 = mybir.dt.float32
BF16 = mybir.dt.bfloat16
F16 = mybir.dt.float16

BF = ml_dtypes.bfloat16

_prog_cache = {}


def _split_excess_waits(nc, limit=1):
    """This container's walrus rejects >1 sem-wait per instruction: move
    excess waits onto NoOps inserted just before, on the same engine."""
    n_new = 0
    for blk in nc.m.functions[0].blocks:
        out = []
        for inst in blk.instructions:
            si = inst.sync_info
            if si is not None and si.on_wait and len(si.on_wait) > limit:
                waits = list(si.on_wait)
                head, tail = waits[:-limit], waits[-limit:]
                while head:
                    chunk, head = head[:limit], head[limit:]
                    nop = mybir.InstNoOp(name=f"waitsplit_nop_{n_new}", ins=[], outs=[])
                    n_new += 1
                    nop.engine = inst.engine
                    nop.sync_info = mybir.SyncInfo(on_wait=chunk, on_update=[])
                    out.append(nop)
                inst.sync_info = mybir.SyncInfo(on_wait=tail, on_update=list(si.on_update))
            out.append(inst)
        if len(out) != len(blk.instructions):
            blk.instructions[:] = out
    return n_new


def _bcast_free(ap, reps):
    """Append a step-0 free dim of size `reps` to an AP (per-element broadcast)."""
    return bass.AP(tensor=ap.tensor, offset=ap.offset,
                   ap=[list(p) for p in ap.ap] + [[0, reps]])


def _build_program(with_batt, with_bcomb, with_bih, with_bhh, with_bout):
    nc = bass.Bass("TRN2", target_bir_lowering=False, debug=False)

    din = lambda n, s, d: nc.dram_tensor(n, s, d, kind="ExternalInput").ap()
    dout = lambda n, s, d: nc.dram_tensor(n, s, d, kind="ExternalOutput").ap()

    embT = din("embT", [E, BC], BF16)
    h0 = din("h0", [BC, H], F32)
    h0T = din("h0T", [H, BC], BF16)
    enc = din("enc", [BC, L, H], BF16)
    wattT = din("wattT", [EH, L], BF16)
    wcombT = din("wcombT", [EH, E], BF16)
    wihT = din("wihT", [E, G3], BF16)
    whhT = din("whhT", [H, G3], BF16)
    woutT = din("woutT", [H, VP], BF16)
    batt = din("batt", [1, L], F32) if with_batt else None
    bcomb = din("bcomb", [1, E], F32) if with_bcomb else None
    bih = din("bih", [1, G3], F32) if with_bih else None
    bhh = din("bhh", [1, G3], F32) if with_bhh else None
    bout = din("bout", [1, VP], F32) if with_bout else None

    y_cap = dout("y_cap", [BC, VP], F32)
    h_new = dout("h_new", [BC, H], F32)
    attn_w = dout("attn_w", [BC, L], F32)

    EXP = mybir.ActivationFunctionType.Exp
    LN = mybir.ActivationFunctionType.Ln
    SIG = mybir.ActivationFunctionType.Sigmoid
    TANH = mybir.ActivationFunctionType.Tanh
    RELU = mybir.ActivationFunctionType.Relu
    X = mybir.AxisListType.X
    MAX = mybir.AluOpType.max
    ADD = mybir.AluOpType.add
    SUB = mybir.AluOpType.subtract
    MUL = mybir.AluOpType.mult

    with tile.TileContext(nc) as tc:
        with tc.tile_pool(name="persist", bufs=1) as persist:
            # survives both phases
            ident32 = persist.tile([P, P], F32)
            make_identity(nc, ident32)
            identbf = persist.tile([P, P], BF16)
            make_identity(nc, identbf)
            hT_sb = persist.tile([P, 4, NB, P], BF16)      # h_new^T k-tiles
            stile = persist.tile([P, NB, NCH], F32)        # per-chunk exp sums
            negC = persist.tile([P, 1], F32)               # exp bias = -C
            nc.vector.memset(negC, -EXP_C)
            if with_batt or with_bcomb or with_bih or with_bhh or with_bout:
                ones1 = persist.tile([1, P], F32)
                nc.vector.memset(ones1, 1.0)

            # ---------------- Phase 1: attention + GRU ----------------
            with tc.tile_pool(name="p1const", bufs=1) as p1c, \
                 tc.tile_pool(name="p1enc", bufs=2) as p1enc, \
                 tc.tile_pool(name="p1w", bufs=2) as p1w, \
                 tc.tile_pool(name="ps1", bufs=2, space="PSUM") as ps1:

                wattT_sb = p1c.tile([P, 6, L], BF16)
                nc.sync.dma_start(out=wattT_sb, in_=wattT.rearrange("(k p) n -> p k n", p=P))
                wcombT_sb = p1c.tile([P, 6, E], BF16)
                nc.sync.dma_start(out=wcombT_sb, in_=wcombT.rearrange("(k p) n -> p k n", p=P))
                wihT_sb = p1c.tile([P, 2, G3], BF16)
                nc.sync.dma_start(out=wihT_sb, in_=wihT.rearrange("(k p) n -> p k n", p=P))
                whhT_sb = p1c.tile([P, 4, G3], BF16)
                nc.sync.dma_start(out=whhT_sb, in_=whhT.rearrange("(k p) n -> p k n", p=P))
                embT_sb = p1c.tile([P, 2, BC], BF16)
                nc.sync.dma_start(out=embT_sb, in_=embT.rearrange("(k p) b -> p k b", p=P))
                h0T_sb = p1c.tile([P, 4, BC], BF16)
                nc.sync.dma_start(out=h0T_sb, in_=h0T.rearrange("(k p) b -> p k b", p=P))
                h0_sb = p1c.tile([P, NB, H], F32)
                nc.sync.dma_start(out=h0_sb, in_=h0.rearrange("(b p) h -> p b h", p=P))
                batt_sb = bcomb_sb = bih_sb = bhh_sb = None
                if with_batt:
                    batt_sb = p1c.tile([1, L], F32)
                    nc.sync.dma_start(out=batt_sb, in_=batt)
                if with_bcomb:
                    bcomb_sb = p1c.tile([1, E], F32)
                    nc.sync.dma_start(out=bcomb_sb, in_=bcomb)
                if with_bih:
                    bih_sb = p1c.tile([1, G3], F32)
                    nc.sync.dma_start(out=bih_sb, in_=bih)
                if with_bhh:
                    bhh_sb = p1c.tile([1, G3], F32)
                    nc.sync.dma_start(out=bhh_sb, in_=bhh)

                for ib in range(NB):
                    cols = bass.ts(ib, P)   # batch columns of this block in *T tiles
                    rows = bass.ts(ib, P)

                    # --- attention scores S = att_in @ W_att.T (+ b_att) ---
                    ps_S = ps1.tile([P, L], F32, tag="sm")
                    for k in range(6):
                        lhsT = embT_sb[:, k, cols] if k < 2 else h0T_sb[:, k - 2, cols]
                        nc.tensor.matmul(ps_S, lhsT, wattT_sb[:, k, :],
                                         start=(k == 0), stop=(k == 5 and not with_batt))
                    if with_batt:
                        nc.tensor.matmul(ps_S, ones1, batt_sb, start=False, stop=True)

                    # --- softmax over L (free dim) ---
                    nmax = p1w.tile([P, 1], F32)
                    nc.vector.tensor_reduce(out=nmax, in_=ps_S, axis=X, op=MAX, negate=True)
                    ew = p1w.tile([P, L], F32)
                    ssum = p1w.tile([P, 1], F32)
                    nc.scalar.activation(out=ew, in_=ps_S, func=EXP, bias=nmax, accum_out=ssum)
                    rinv = p1w.tile([P, 1], F32)
                    nc.vector.reciprocal(out=rinv, in_=ssum)
                    wn = p1w.tile([P, L], F32)
                    nc.vector.tensor_scalar_mul(wn, ew, rinv)
                    nc.sync.dma_start(out=attn_w[rows, :], in_=wn)

                    # --- attn_applied = sum_l w[:,l] * enc[:,l,:]  via diag matmuls ---
                    ps_aa = ps1.tile([P, H], F32, tag="aa")
                    for j in range(8):          # chunks of 8 attention slots
                        enc_t = p1enc.tile([P, 8, H], BF16)
                        nc.sync.dma_start(out=enc_t, in_=enc[rows, j * 8:(j + 1) * 8, :])
                        diag = p1w.tile([P, 8, P], BF16)
                        for jl in range(8):
                            l = j * 8 + jl
                            nc.vector.tensor_scalar(diag[:, jl, :], identbf,
                                                    wn[:, l:l + 1], None, op0=MUL)
                            nc.tensor.matmul(ps_aa, diag[:, jl, :], enc_t[:, jl, :],
                                             start=(l == 0), stop=(l == 63))
                    aa = p1w.tile([P, H], F32)
                    nc.scalar.copy(out=aa, in_=ps_aa)

                    # aa^T (bf16 k-tiles)
                    aaT = p1w.tile([P, 4, P], BF16)
                    for t in range(4):
                        ps_tr = ps1.tile([P, P], F32, tag="sm")
                        nc.tensor.transpose(ps_tr, aa[:, bass.ts(t, P)], ident32)
                        nc.vector.tensor_copy(out=aaT[:, t, :], in_=ps_tr)

                    # --- y = relu([embedded, aa] @ W_comb.T (+ b_comb)) ---
                    ps_y = ps1.tile([P, E], F32, tag="sm")
                    for k in range(6):
                        lhsT = embT_sb[:, k, cols] if k < 2 else aaT[:, k - 2, :]
                        nc.tensor.matmul(ps_y, lhsT, wcombT_sb[:, k, :],
                                         start=(k == 0), stop=(k == 5 and not with_bcomb))
                    if with_bcomb:
                        nc.tensor.matmul(ps_y, ones1, bcomb_sb, start=False, stop=True)
                    ybf = p1w.tile([P, E], BF16)
                    nc.scalar.activation(out=ybf, in_=ps_y, func=RELU)
                    yT = p1w.tile([P, 2, P], BF16)
                    for t in range(2):
                        ps_trb = ps1.tile([P, P], BF16, tag="sm")
                        nc.tensor.transpose(ps_trb, ybf[:, bass.ts(t, P)], identbf)
                        nc.vector.tensor_copy(out=yT[:, t, :], in_=ps_trb)

                    # --- GRU gates: gi = y @ W_ih.T, gh = h0 @ W_hh.T (per 512-slice) ---
                    # r,z slices: PE accumulates gi+gh into one PSUM group.
                    rzs = p1w.tile([P, 1024], F32)      # sigmoid(gi+gh) for r,z
                    for n3 in range(2):
                        nsl = bass.ts(n3, VT)
                        ps_g = ps1.tile([P, VT], F32, tag="gg")
                        for k in range(2):
                            nc.tensor.matmul(ps_g, yT[:, k, :], wihT_sb[:, k, nsl],
                                             start=(k == 0), stop=False)
                        if with_bih:
                            nc.tensor.matmul(ps_g, ones1, bih_sb[:, nsl], start=False, stop=False)
                        for k in range(4):
                            nc.tensor.matmul(ps_g, h0T_sb[:, k, cols], whhT_sb[:, k, nsl],
                                             start=False, stop=(k == 3 and not with_bhh))
                        if with_bhh:
                            nc.tensor.matmul(ps_g, ones1, bhh_sb[:, nsl], start=False, stop=True)
                        nc.scalar.activation(out=rzs[:, nsl], in_=ps_g, func=SIG)
                    # n slice: gi and gh kept separate (r gates gh_n before the add)
                    nsl = bass.ts(2, VT)
                    ps_gi = ps1.tile([P, VT], F32, tag="gi", bufs=1)
                    for k in range(2):
                        nc.tensor.matmul(ps_gi, yT[:, k, :], wihT_sb[:, k, nsl],
                                         start=(k == 0), stop=(k == 1 and not with_bih))
                    if with_bih:
                        nc.tensor.matmul(ps_gi, ones1, bih_sb[:, nsl], start=False, stop=True)
                    ps_gh = ps1.tile([P, VT], F32, tag="gh", bufs=1)
                    for k in range(4):
                        nc.tensor.matmul(ps_gh, h0T_sb[:, k, cols], whhT_sb[:, k, nsl],
                                         start=(k == 0), stop=(k == 3 and not with_bhh))
                    if with_bhh:
                        nc.tensor.matmul(ps_gh, ones1, bhh_sb[:, nsl], start=False, stop=True)
                    rgh = p1w.tile([P, VT], F32)
                    nc.vector.tensor_tensor(rgh, rzs[:, 0:VT], ps_gh, op=MUL)
                    npre = p1w.tile([P, VT], F32)
                    nc.vector.tensor_tensor(npre, ps_gi, rgh, op=ADD)
                    nt = p1w.tile([P, VT], F32)
                    nc.scalar.activation(out=nt, in_=npre, func=TANH)
                    hmn = p1w.tile([P, VT], F32)
                    nc.vector.tensor_tensor(hmn, h0_sb[:, ib, :], nt, op=SUB)
                    zh = p1w.tile([P, VT], F32)
                    nc.vector.tensor_tensor(zh, rzs[:, VT:2 * VT], hmn, op=MUL)
                    hn = p1w.tile([P, VT], F32)
                    nc.vector.tensor_tensor(hn, zh, nt, op=ADD)
                    nc.sync.dma_start(out=h_new[rows, :], in_=hn)
                    for t in range(4):
                        ps_tr = ps1.tile([P, P], F32, tag="sm")
                        nc.tensor.transpose(ps_tr, hn[:, bass.ts(t, P)], ident32)
                        nc.vector.tensor_copy(out=hT_sb[:, t, ib, :], in_=ps_tr)

            # ---------------- Phase 2: vocab projection + log_softmax ----------------
            with tc.tile_pool(name="p16pool", bufs=1) as p16pool, \
                 tc.tile_pool(name="p2w", bufs=2) as p2w, \
                 tc.tile_pool(name="p2out", bufs=2) as p2out, \
                 tc.tile_pool(name="ps2", bufs=4, space="PSUM") as ps2:

                p16 = p16pool.tile([P, NB, VP], F16)
                woutT_r = woutT.rearrange("(k p) v -> p k v", p=P)

                for ch in range(NCH):
                    vsl = slice(ch * 2 * VT, (ch + 1) * 2 * VT)
                    wt = p2w.tile([P, 4, 2 * VT], BF16)
                    nc.sync.dma_start(out=wt, in_=woutT_r[:, :, vsl])
                    for ib in range(NB):
                        pst = ps2.tile([P, 2 * VT], F32, tag="mm")
                        for k in range(4):
                            for nn in range(2):
                                nc.tensor.matmul(
                                    pst[:, bass.ts(nn, VT)],
                                    hT_sb[:, k, ib, :],
                                    wt[:, k, bass.ts(nn, VT)],
                                    start=(k == 0), stop=(k == 3 and not with_bout))
                        if with_bout:
                            bo = p2w.tile([1, 2 * VT], F32, tag="bo")
                            nc.sync.dma_start(out=bo, in_=bout[:, vsl])
                            for nn in range(2):
                                nc.tensor.matmul(pst[:, bass.ts(nn, VT)], ones1,
                                                 bo[:, bass.ts(nn, VT)],
                                                 start=False, stop=True)
                        nc.scalar.activation(out=p16[:, ib, vsl], in_=pst, func=EXP,
                                             bias=negC, accum_out=stile[:, ib, ch:ch + 1])

                for ib in range(NB):
                    srow = p2w.tile([P, 1], F32, tag="srow")
                    nc.vector.tensor_reduce(out=srow, in_=stile[:, ib, :], axis=X, op=ADD)
                    # padded vocab columns contribute exp(0 - C) each; remove them
                    nc.vector.tensor_scalar_add(srow, srow, -VPAD * math.exp(-EXP_C))
                    rinv2 = p2w.tile([P, 1], F32, tag="rinv2")
                    nc.vector.reciprocal(out=rinv2, in_=srow)
                    for oc in range(16):
                        osl = slice(oc * 2048, (oc + 1) * 2048)
                        ot = p2out.tile([P, 2048], F32)
                        nc.scalar.activation(out=ot, in_=p16[:, ib, osl], func=LN,
                                             scale=rinv2)
                        nc.sync.dma_start(out=y_cap[bass.ts(ib, P), osl], in_=ot)

    _split_excess_waits(nc)
    return nc


def _host_prep(inputs):
    idx = np.asarray(inputs["input"])[0]
    emb = np.asarray(inputs["emb"], dtype=np.float32)
    hidden = np.asarray(inputs["hidden"], dtype=np.float32)[0]          # [B,H]
    enc = np.asarray(inputs["encoder_outputs"], dtype=np.float32)       # [B,L,H]
    W_att = np.asarray(inputs["W_att"], dtype=np.float32)
    W_comb = np.asarray(inputs["W_comb"], dtype=np.float32)
    W_ih = np.asarray(inputs["W_ih"], dtype=np.float32)
    W_hh = np.asarray(inputs["W_hh"], dtype=np.float32)
    W_out = np.asarray(inputs["W_out"], dtype=np.float32)
    b_att = np.asarray(inputs["b_att"], dtype=np.float32)
    b_comb = np.asarray(inputs["b_comb"], dtype=np.float32)
    b_ih = np.asarray(inputs["b_ih"], dtype=np.float32)
    b_hh = np.asarray(inputs["b_hh"], dtype=np.float32)
    b_out = np.asarray(inputs["b_out"], dtype=np.float32)

    emb_g = emb[idx]                                      # [B, E] gather
    flags = (bool(b_att.any()), bool(b_comb.any()), bool(b_ih.any()),
             bool(b_hh.any()), bool(b_out.any()))

    wattT = np.ascontiguousarray(W_att.T).astype(BF)      # [EH, L]
    wcombT = np.ascontiguousarray(W_comb.T).astype(BF)    # [EH, E]
    wihT = np.ascontiguousarray(W_ih.T).astype(BF)        # [E, G3]
    whhT = np.ascontiguousarray(W_hh.T).astype(BF)        # [H, G3]
    woutT = np.zeros((H, VP), dtype=BF)
    woutT[:, :V] = W_out.T.astype(BF)
    bout_pad = np.zeros((1, VP), dtype=np.float32)
    bout_pad[0, :V] = b_out

    in_maps = []
    for c in range(NCORES):
        sl = slice(c * BC, (c + 1) * BC)
        m = {
            "embT": np.ascontiguousarray(emb_g[sl].T).astype(BF),
            "h0": np.ascontiguousarray(hidden[sl]),
            "h0T": np.ascontiguousarray(hidden[sl].T).astype(BF),
            "enc": np.ascontiguousarray(enc[sl]).astype(BF),
            "wattT": wattT, "wcombT": wcombT, "wihT": wihT,
            "whhT": whhT, "woutT": woutT,
        }
        if flags[0]:
            m["batt"] = b_att[None, :]
        if flags[1]:
            m["bcomb"] = b_comb[None, :]
        if flags[2]:
            m["bih"] = b_ih[None, :]
        if flags[3]:
            m["bhh"] = b_hh[None, :]
        if flags[4]:
            m["bout"] = bout_pad
        in_maps.append(m)
    return in_maps, flags


def _get_program(flags):
    if flags not in _prog_cache:
        _prog_cache[flags] = _build_program(*flags)
    return _prog_cache[flags]


def kernel(**inputs):
    assert int(inputs["batch_size"]) == B
    in_maps, flags = _host_prep(inputs)
    nc = _get_program(flags)
    res = run_bass_kernel_spmd(nc, in_maps, core_ids=list(range(NCORES)))

    y_cap = np.empty((B, V), dtype=np.float32)
    h_new = np.empty((1, B, H), dtype=np.float32)
    attn_w = np.empty((B, L), dtype=np.float32)
    for c in range(NCORES):
        sl = slice(c * BC, (c + 1) * BC)
        r = res.results[c]
        y_cap[sl] = r["y_cap"][:, :V]
        h_new[0, sl] = r["h_new"]
        attn_w[sl] = r["attn_w"]
    return y_cap, h_new, attn_w
